# revision 16
# baseline (speedup 1.0000x reference)
"""Channel-attention transformer block on 8 Trainium2 NeuronCores.

Reference semantics (b=8, l=4096, c=512, h=8 heads carved from the
*sequence* axis, head_pos = l % 8):
    qkv = x @ w_qkv.T ; split q,k,v per head  (each (lh=512, c=512))
    attn = softmax((q.T @ k) / 8, axis=-1)    # (c, c) channel attention
    y.T  = attn @ v.T                         # (c, lh)
    out  = y @ w_out.T + b_out
Sharding: data-parallel over batch - core i handles batch i.

Layout (same math as the fp32r baseline, see kernel_baseline.py):
the l axis is permuted on the host so each head's 512 rows are
contiguous, x is shipped transposed (c, l); Q,K come out of matmuls in
natural (l, c) layout and V^T in (c, l); scores are computed transposed
(S^T = K^T Q) so the softmax sum lands on the partition dim where a
ones-column glued onto the AV rhs computes it; 1/denominator is fused
into the AV evacuation; the out-projection consumes y^T directly.

Perf changes vs the baseline (205 us):
- bf16 matmul path end to end (fp32 PSUM): enables FWL fast weight
  loads (hidden LDWEIGHTS), halves input DMA bytes, bf16 out + host
  upcast halves output DMA.
- PSUM evacuations split across Vector AND Scalar engines (both ~95
  G elem/s on PSUM reads; baseline put ~everything on Vector): q,k and
  out-copies on Vector; v, exp, and the normalize-fused y evacuation
  (activation Copy with per-partition scale=1/denom) on Scalar.
- 3 DMA rails (sync+scalar HWDGE, gpsimd SWDGE) with critical-first
  order: head0 x and wq land in parallel, first matmul ~5 us earlier.
- ~20 warm-up matmuls on a zeroed scratch tile right after the
  preamble: the PE HAM clock-gate un-throttles (1.2->2.4 GHz) during
  the input DMA instead of 20+ us into real work.
- Head stages software-pipelined (P=proj, S=scores, A=AV, O=out-proj
  emitted as P0 S0 | P1 A0 S1 O0 | ...) so exp/evacuation latency of
  head h hides under projections of head h+1 and the PE never stalls.
"""

import numpy as np
import ml_dtypes

import concourse.bass as bass
import concourse.mybir as mybir
import concourse.tile as tile
from concourse.bass_utils import run_bass_kernel_spmd

B = 8
L = 4096
C = 512
HEADS = 8
LH = L // HEADS  # 512
SCALE = 64 ** -0.5  # DIM_HEAD ** -0.5 from the reference
N_CORES = 8
P = 128
KC = C // P  # 4 contraction chunks of 128
F32 = mybir.dt.float32
BF16 = mybir.dt.bfloat16
NPBF16 = ml_dtypes.bfloat16

WARM_MMS = 20   # PE warm-up dummies (HAM un-throttle needs ~3.4us busy)
WARM_N = 256
NY1 = 258  # AV split: 2 denominator cols + 256 v cols (psum bank = 512 fp32)


def _split_wide_waits(nc, max_waits=1):
    """This container's walrus build rejects instructions carrying more than
    ~1 sync wait ("Too many sync wait commands", e.g. in the S3_LW lowering
    of a fused matmul). Hoist surplus waits onto same-engine nops inserted
    immediately before the offending instruction - the engine stalls at the
    same point in its stream, so scheduling semantics are unchanged."""
    for f in nc.m.functions:
        for bb in f.blocks:
            snapshot = list(bb.instructions)
            if not any(
                inst.sync_info and inst.sync_info.on_wait
                and len(inst.sync_info.on_wait) > max_waits
                for inst in snapshot
            ):
                continue
            new = []
            for inst in snapshot:
                si = inst.sync_info
                waits = list(si.on_wait) if si and si.on_wait else []
                if len(waits) > max_waits:
                    for w in waits[:-max_waits]:
                        nop = nc.engines[inst.engine].nop(nofuse=True).ins
                        cur = nc.cur_bb.bb.instructions
                        assert cur[-1] is nop
                        cur.pop()  # re-homed below, right before `inst`
                        nop.sync_info = mybir.SyncInfo(on_wait=[w], on_update=[])
                        new.append(nop)
                    inst.sync_info = mybir.SyncInfo(
                        on_wait=waits[-max_waits:],
                        on_update=list(si.on_update) if si.on_update else [],
                    )
                new.append(inst)
            bb.instructions = new


def _emit(ctx, tc, xt, wq_t, wk_t, wv_t, wout_t, out):
    """Per-core program. DRAM APs: xt (C, L) bf16, wq_t/wk_t/wv_t/wout_t
    (C, C) bf16 (q pre-scaled), out (L, C) bf16."""
    nc = tc.nc
    EXP = mybir.ActivationFunctionType.Exp

    # all DRAM inputs are host-prearranged to partition-major layouts so
    # every DMA coalesces to 128 descriptors of 4KB (descriptor count, not
    # bytes, is what gates the DGE rails: ~33ns/desc HWDGE, ~5ns/desc SWDGE)
    xt_r = xt      # (P, HEADS, KC, LH)
    wq_r = wq_t    # (P, KC, C)
    wk_r = wk_t
    wv_r = wv_t
    wout_r = wout_t

    consts = ctx.enter_context(tc.tile_pool(name="consts", bufs=1))
    xt_pool = ctx.enter_context(tc.tile_pool(name="xt", bufs=HEADS))
    q_pool = ctx.enter_context(tc.tile_pool(name="q", bufs=2))
    k_pool = ctx.enter_context(tc.tile_pool(name="k", bufs=2))
    vt_pool = ctx.enter_context(tc.tile_pool(name="vt", bufs=2))
    exp_pool = ctx.enter_context(tc.tile_pool(name="exp", bufs=2))
    y_pool = ctx.enter_context(tc.tile_pool(name="y", bufs=2))
    out_pool = ctx.enter_context(tc.tile_pool(name="out", bufs=8))
    recip_pool = ctx.enter_context(tc.tile_pool(name="recip", bufs=8))

    wq = consts.tile([P, KC, C], BF16)
    wk = consts.tile([P, KC, C], BF16)
    wv = consts.tile([P, KC, C], BF16)
    wout = consts.tile([P, KC, C], BF16)
    warm = consts.tile([P, WARM_N], BF16)

    xth = [xt_pool.tile([P, KC, LH], BF16, tag="xth", name=f"xth{h}")
           for h in range(HEADS)]

    # --- startup DMA. wq (needed first) split across the two HWDGE rails;
    # the SWDGE/gpsimd rail streams xth0 + the other weights in consumption
    # order; late x heads ride the HWDGE rails where there is slack.
    # Descriptor lines capped at 2KB (1024 bf16) - 4KB lines hard-fault the
    # device (NRT_EXEC_UNIT_UNRECOVERABLE). ---
    MD = dict(max_dma_last_dim=1024)
    nc.gpsimd.dma_start(wq[:], wq_r[:], **MD)
    nc.gpsimd.dma_start(xth[0][:], xt_r[:, 0, :, :], **MD)
    nc.vector.memset(warm[:], 0.0)
    nc.gpsimd.dma_start(wk[:], wk_r[:], **MD)
    nc.gpsimd.dma_start(wv[:], wv_r[:], **MD)
    nc.gpsimd.dma_start(xth[1][:], xt_r[:, 1, :, :], **MD)
    nc.gpsimd.dma_start(wout[:], wout_r[:], **MD)
    nc.gpsimd.dma_start(xth[2][:], xt_r[:, 2, :, :], **MD)
    nc.gpsimd.dma_start(xth[3][:], xt_r[:, 3, :, :], **MD)
    nc.sync.dma_start(xth[4][:], xt_r[:, 4, :, :], **MD)
    nc.scalar.dma_start(xth[5][:], xt_r[:, 5, :, :], **MD)
    nc.sync.dma_start(xth[6][:], xt_r[:, 6, :, :], **MD)
    nc.scalar.dma_start(xth[7][:], xt_r[:, 7, :, :], **MD)

    # --- PE warm-up: dummy matmuls on zeros keep the PE busy from the
    # end of the preamble so the HAM clock-gate lifts (1.2 -> 2.4 GHz)
    # before the real matmuls start. Own PSUM pool, closed before pp_mm
    # opens so no bank is wasted. ---
    with tc.tile_pool(name="warmp", bufs=1, space="PSUM") as wpool:
        wp = wpool.tile([P, WARM_N], F32)
        for _ in range(WARM_MMS):
            nc.tensor.matmul(wp[:], warm[:, 0:P], warm[:],
                             start=True, stop=True)

    pp_mm = ctx.enter_context(tc.tile_pool(name="pp_mm", bufs=8, space="PSUM"))

    qkv_tiles = {}
    ex_tiles = {}
    y_tiles = {}

    def emit_P(h):
        # projections: Q,K natural (l, c); V^T (c, l) with ones cols
        xh = xth[h]
        q = q_pool.tile([P, KC, C], BF16)
        k = k_pool.tile([P, KC, C], BF16)
        vt = vt_pool.tile([P, KC, LH + 2], BF16)
        nc.gpsimd.memset(vt[:, :, 0:2], 1.0)
        for m in range(KC):  # l' strips of 128
            pq = pp_mm.tile([P, C], F32, tag="mm")
            for ko in range(KC):
                nc.tensor.matmul(pq[:], xh[:, ko, bass.ts(m, P)],
                                 wq[:, ko, :],
                                 start=(ko == 0), stop=(ko == KC - 1))
            nc.vector.tensor_copy(q[:, m, :], pq[:])
        for m in range(KC):
            pk = pp_mm.tile([P, C], F32, tag="mm")
            for ko in range(KC):
                nc.tensor.matmul(pk[:], xh[:, ko, bass.ts(m, P)],
                                 wk[:, ko, :],
                                 start=(ko == 0), stop=(ko == KC - 1))
            nc.vector.tensor_copy(k[:, m, :], pk[:])
        for m in range(KC):  # c_v strips of 128
            pv = pp_mm.tile([P, LH], F32, tag="mm")
            for ko in range(KC):
                nc.tensor.matmul(pv[:], wv[:, ko, bass.ts(m, P)],
                                 xh[:, ko, :],
                                 start=(ko == 0), stop=(ko == KC - 1))
            nc.scalar.copy(vt[:, m, 2:LH + 2], pv[:])
        qkv_tiles[h] = (q, k, vt)

    def emit_S(h):
        # scores transposed + exp: S^T[d, c] = sum_l K[l,d] Q[l,c]
        q, k, _ = qkv_tiles[h]
        ex = exp_pool.tile([P, KC, C], BF16)
        for ds_ in range(KC):  # d strips of 128
            ps = pp_mm.tile([P, C], F32, tag="mm")
            for m in range(KC):  # contraction over l' chunks
                nc.tensor.matmul(ps[:], k[:, m, bass.ts(ds_, P)],
                                 q[:, m, :],
                                 start=(m == 0), stop=(m == KC - 1))
            nc.scalar.activation(ex[:, ds_, :], ps[:], EXP)
        ex_tiles[h] = ex

    def emit_A(h):
        # AV with fused denominator (rhs cols 0,1 of py1 are ones);
        # normalization fused into the Scalar-engine evacuation via
        # activation(Copy, scale=1/denom per partition).
        _, _, vt = qkv_tiles[h]
        ex = ex_tiles[h]
        y = y_pool.tile([P, KC, LH], BF16)
        for cs in range(KC):  # c strips of 128
            py1 = pp_mm.tile([P, NY1], F32, tag="mm")
            py2 = pp_mm.tile([P, LH + 2 - NY1], F32, tag="mm")
            for ko in range(KC):  # contraction over d chunks
                nc.tensor.matmul(py1[:], ex[:, ko, bass.ts(cs, P)],
                                 vt[:, ko, 0:NY1],
                                 start=(ko == 0), stop=(ko == KC - 1))
            for ko in range(KC):
                nc.tensor.matmul(py2[:], ex[:, ko, bass.ts(cs, P)],
                                 vt[:, ko, NY1:LH + 2],
                                 start=(ko == 0), stop=(ko == KC - 1))
            rc = recip_pool.tile([P, 1], F32)
            nc.vector.reciprocal(rc[:], py1[:, 0:1])
            # split the normalize-evacuation across both engines so the
            # last head's y is ready ~2x sooner for the out-projection
            nc.scalar.mul(y[:, cs, 0:NY1 - 2], py1[:, 2:NY1], rc[:])
            nc.vector.tensor_scalar_mul(y[:, cs, NY1 - 2:LH], py2[:], rc[:])
        y_tiles[h] = y

    def emit_O(h):
        # out projection: out[l, co] = sum_c y^T[c, l] woutT[c, co]
        y = y_tiles[h]
        for m in range(KC):  # l' strips of 128
            po = pp_mm.tile([P, C], F32, tag="mm")
            for ko in range(KC):
                nc.tensor.matmul(po[:], y[:, ko, bass.ts(m, P)],
                                 wout[:, ko, :],
                                 start=(ko == 0), stop=(ko == KC - 1))
            ot = out_pool.tile([P, C], BF16)
            if m % 2 == 0:
                nc.vector.tensor_copy(ot[:], po[:])
                nc.sync.dma_start(out[bass.ds(h * LH + m * P, P), :], ot[:])
            else:
                nc.scalar.copy(ot[:], po[:])
                nc.scalar.dma_start(out[bass.ds(h * LH + m * P, P), :], ot[:])

    # software pipeline: exp/evac latency of head h hides under head h+1
    emit_P(0)
    emit_S(0)
    for h in range(1, HEADS):
        emit_P(h)
        emit_A(h - 1)
        emit_S(h)
        emit_O(h - 1)
    emit_A(HEADS - 1)
    emit_O(HEADS - 1)


def _build_program():
    nc = bass.Bass(trn_type="TRN2", target_bir_lowering=False, debug=False,
                   num_devices=N_CORES)
    xt = nc.dram_tensor("xt", [P, HEADS, KC, LH], BF16,
                        kind="ExternalInput").ap()
    wq_t = nc.dram_tensor("wq_t", [P, KC, C], BF16, kind="ExternalInput").ap()
    wk_t = nc.dram_tensor("wk_t", [P, KC, C], BF16, kind="ExternalInput").ap()
    wv_t = nc.dram_tensor("wv_t", [P, KC, C], BF16, kind="ExternalInput").ap()
    wout_t = nc.dram_tensor("wout_t", [P, KC, C], BF16,
                            kind="ExternalInput").ap()
    out = nc.dram_tensor("out", [L, C], BF16, kind="ExternalOutput").ap()

    from contextlib import ExitStack
    with tile.TileContext(nc) as tc:
        with ExitStack() as ctx:
            _emit(ctx, tc, xt, wq_t, wk_t, wv_t, wout_t, out)
    _split_wide_waits(nc)
    return nc


def _part_major(w_t):
    """(C, C_out) -> (P, KC, C_out): row ko*P+p lands at [p, ko, :] so each
    DMA partition line is one contiguous 4KB chunk."""
    return np.ascontiguousarray(
        w_t.reshape(KC, P, C).transpose(1, 0, 2)).astype(NPBF16)


def _host_inputs(x, w_qkv, w_out):
    """Per-core input maps. Permute l so head h owns rows [h*512, (h+1)*512)
    (original row i*8+h -> permuted row h*512+i), transpose to (c, l), then
    rearrange everything partition-major for single-descriptor DMA lines."""
    wqkv_t = np.ascontiguousarray(w_qkv.T).astype(np.float32)  # (c, 3c)
    wq_t = _part_major(wqkv_t[:, 0:C] * SCALE)
    wk_t = _part_major(wqkv_t[:, C:2 * C])
    wv_t = _part_major(wqkv_t[:, 2 * C:3 * C])
    wout_t = _part_major(w_out.T.astype(np.float32))
    in_maps = []
    for b in range(B):
        xb = x[b]  # (L, C); row l = i*8 + h
        x_perm = xb.reshape(LH, HEADS, C).transpose(1, 0, 2).reshape(L, C)
        xt = np.ascontiguousarray(x_perm.T)  # (c, l)
        # (ko*P+p, h*LH+l') -> (p, h, ko, l')
        xt_hw = np.ascontiguousarray(
            xt.reshape(KC, P, HEADS, LH).transpose(1, 2, 0, 3)).astype(NPBF16)
        in_maps.append({"xt": xt_hw, "wq_t": wq_t, "wk_t": wk_t,
                        "wv_t": wv_t, "wout_t": wout_t})
    return in_maps


def _unpermute(out_perm):
    """(L, C) with rows grouped by head -> original row order i*8+h."""
    return out_perm.reshape(HEADS, LH, C).transpose(1, 0, 2).reshape(L, C)


def kernel(x, w_qkv, w_out, b_out, _run_kwargs=None):
    x = np.asarray(x, dtype=np.float32)
    w_qkv = np.asarray(w_qkv, dtype=np.float32)
    w_out = np.asarray(w_out, dtype=np.float32)
    b_out = np.asarray(b_out, dtype=np.float32)

    nc = _build_program()
    in_maps = _host_inputs(x, w_qkv, w_out)
    res = run_bass_kernel_spmd(nc, in_maps, list(range(N_CORES)),
                               **(_run_kwargs or {}))
    out = np.empty((B, L, C), dtype=np.float32)
    for b in range(B):
        out[b] = _unpermute(res.results[b]["out"].astype(np.float32))
    out += b_out
    if _run_kwargs:
        kernel.last_result = res
    return out


# revision 18
# speedup vs baseline: 1.0210x; 1.0210x over previous
"""Channel-attention transformer block on 8 Trainium2 NeuronCores.

Reference semantics (b=8, l=4096, c=512, h=8 heads carved from the
*sequence* axis, head_pos = l % 8):
    qkv = x @ w_qkv.T ; split q,k,v per head  (each (lh=512, c=512))
    attn = softmax((q.T @ k) / 8, axis=-1)    # (c, c) channel attention
    y.T  = attn @ v.T                         # (c, lh)
    out  = y @ w_out.T + b_out
Sharding: data-parallel over batch - core i handles batch i.

Layout (same math as the fp32r baseline, see kernel_baseline.py):
the l axis is permuted on the host so each head's 512 rows are
contiguous, x is shipped transposed (c, l); Q,K come out of matmuls in
natural (l, c) layout and V^T in (c, l); scores are computed transposed
(S^T = K^T Q) so the softmax sum lands on the partition dim where a
ones-column glued onto the AV rhs computes it; 1/denominator is fused
into the AV evacuation; the out-projection consumes y^T directly.

Perf changes vs the baseline (205 us):
- bf16 matmul path end to end (fp32 PSUM): enables FWL fast weight
  loads (hidden LDWEIGHTS), halves input DMA bytes, bf16 out + host
  upcast halves output DMA.
- PSUM evacuations split across Vector AND Scalar engines (both ~95
  G elem/s on PSUM reads; baseline put ~everything on Vector): q,k and
  out-copies on Vector; v, exp, and the normalize-fused y evacuation
  (activation Copy with per-partition scale=1/denom) on Scalar.
- 3 DMA rails (sync+scalar HWDGE, gpsimd SWDGE) with critical-first
  order: head0 x and wq land in parallel, first matmul ~5 us earlier.
- ~20 warm-up matmuls on a zeroed scratch tile right after the
  preamble: the PE HAM clock-gate un-throttles (1.2->2.4 GHz) during
  the input DMA instead of 20+ us into real work.
- Head stages software-pipelined (P=proj, S=scores, A=AV, O=out-proj
  emitted as P0 S0 | P1 A0 S1 O0 | ...) so exp/evacuation latency of
  head h hides under projections of head h+1 and the PE never stalls.
"""

import numpy as np
import ml_dtypes

import concourse.bass as bass
import concourse.mybir as mybir
import concourse.tile as tile
from concourse.bass_utils import run_bass_kernel_spmd

B = 8
L = 4096
C = 512
HEADS = 8
LH = L // HEADS  # 512
SCALE = 64 ** -0.5  # DIM_HEAD ** -0.5 from the reference
N_CORES = 8
P = 128
KC = C // P  # 4 contraction chunks of 128
F32 = mybir.dt.float32
BF16 = mybir.dt.bfloat16
NPBF16 = ml_dtypes.bfloat16

WARM_MMS = 24   # PE warm-up dummies (HAM un-throttle needs ~3.4us busy)
WARM_N = 256
NY1 = 258  # AV split: 2 denominator cols + 256 v cols (psum bank = 512 fp32)


def _split_wide_waits(nc, max_waits=1):
    """This container's walrus build rejects instructions carrying more than
    ~1 sync wait ("Too many sync wait commands", e.g. in the S3_LW lowering
    of a fused matmul). Hoist surplus waits onto same-engine nops inserted
    immediately before the offending instruction - the engine stalls at the
    same point in its stream, so scheduling semantics are unchanged."""
    for f in nc.m.functions:
        for bb in f.blocks:
            snapshot = list(bb.instructions)
            if not any(
                inst.sync_info and inst.sync_info.on_wait
                and len(inst.sync_info.on_wait) > max_waits
                for inst in snapshot
            ):
                continue
            new = []
            for inst in snapshot:
                si = inst.sync_info
                waits = list(si.on_wait) if si and si.on_wait else []
                if len(waits) > max_waits:
                    for w in waits[:-max_waits]:
                        nop = nc.engines[inst.engine].nop(nofuse=True).ins
                        cur = nc.cur_bb.bb.instructions
                        assert cur[-1] is nop
                        cur.pop()  # re-homed below, right before `inst`
                        nop.sync_info = mybir.SyncInfo(on_wait=[w], on_update=[])
                        new.append(nop)
                    inst.sync_info = mybir.SyncInfo(
                        on_wait=waits[-max_waits:],
                        on_update=list(si.on_update) if si.on_update else [],
                    )
                new.append(inst)
            bb.instructions = new


def _emit(ctx, tc, xt, wq_t, wk_t, wv_t, wout_t, out):
    """Per-core program. DRAM APs: xt (C, L) bf16, wq_t/wk_t/wv_t/wout_t
    (C, C) bf16 (q pre-scaled), out (L, C) bf16."""
    nc = tc.nc
    EXP = mybir.ActivationFunctionType.Exp

    # all DRAM inputs are host-prearranged to partition-major layouts so
    # every DMA coalesces to 128 descriptors of 4KB (descriptor count, not
    # bytes, is what gates the DGE rails: ~33ns/desc HWDGE, ~5ns/desc SWDGE)
    xt_r = xt      # (P, HEADS, KC, LH)
    wq_r = wq_t    # (P, KC, C)
    wk_r = wk_t
    wv_r = wv_t
    wout_r = wout_t

    consts = ctx.enter_context(tc.tile_pool(name="consts", bufs=1))
    xt_pool = ctx.enter_context(tc.tile_pool(name="xt", bufs=HEADS))
    q_pool = ctx.enter_context(tc.tile_pool(name="q", bufs=2))
    k_pool = ctx.enter_context(tc.tile_pool(name="k", bufs=2))
    vt_pool = ctx.enter_context(tc.tile_pool(name="vt", bufs=2))
    exp_pool = ctx.enter_context(tc.tile_pool(name="exp", bufs=2))
    y_pool = ctx.enter_context(tc.tile_pool(name="y", bufs=2))
    out_pool = ctx.enter_context(tc.tile_pool(name="out", bufs=8))
    recip_pool = ctx.enter_context(tc.tile_pool(name="recip", bufs=8))

    wq = consts.tile([P, KC, C], BF16)
    wk = consts.tile([P, KC, C], BF16)
    wv = consts.tile([P, KC, C], BF16)
    wout = consts.tile([P, KC, C], BF16)
    warm = consts.tile([P, WARM_N], BF16)

    xth = [xt_pool.tile([P, KC, LH], BF16, tag="xth", name=f"xth{h}")
           for h in range(HEADS)]

    # --- startup DMA. wq (needed first) split across the two HWDGE rails;
    # the SWDGE/gpsimd rail streams xth0 + the other weights in consumption
    # order; late x heads ride the HWDGE rails where there is slack.
    # Descriptor lines capped at 2KB (1024 bf16) - 4KB lines hard-fault the
    # device (NRT_EXEC_UNIT_UNRECOVERABLE). ---
    MD = dict(max_dma_last_dim=1024)
    nc.gpsimd.dma_start(wq[:], wq_r[:], **MD)
    nc.sync.dma_start(xth[0][:, 0:2, :], xt_r[:, 0, 0:2, :], **MD)
    nc.scalar.dma_start(xth[0][:, 2:KC, :], xt_r[:, 0, 2:KC, :], **MD)
    nc.vector.memset(warm[:], 0.0)
    nc.gpsimd.dma_start(wk[:], wk_r[:], **MD)
    nc.gpsimd.dma_start(wv[:], wv_r[:], **MD)
    nc.gpsimd.dma_start(xth[1][:], xt_r[:, 1, :, :], **MD)
    nc.gpsimd.dma_start(wout[:], wout_r[:], **MD)
    nc.gpsimd.dma_start(xth[2][:], xt_r[:, 2, :, :], **MD)
    nc.gpsimd.dma_start(xth[3][:], xt_r[:, 3, :, :], **MD)
    nc.sync.dma_start(xth[4][:], xt_r[:, 4, :, :], **MD)
    nc.scalar.dma_start(xth[5][:], xt_r[:, 5, :, :], **MD)
    nc.sync.dma_start(xth[6][:], xt_r[:, 6, :, :], **MD)
    nc.scalar.dma_start(xth[7][:], xt_r[:, 7, :, :], **MD)

    # --- PE warm-up: dummy matmuls on zeros keep the PE busy from the
    # end of the preamble so the HAM clock-gate lifts (1.2 -> 2.4 GHz)
    # before the real matmuls start. Own PSUM pool, closed before pp_mm
    # opens so no bank is wasted. ---
    with tc.tile_pool(name="warmp", bufs=1, space="PSUM") as wpool:
        wp = wpool.tile([P, WARM_N], F32)
        for _ in range(WARM_MMS):
            nc.tensor.matmul(wp[:], warm[:, 0:P], warm[:],
                             start=True, stop=True)

    pp_mm = ctx.enter_context(tc.tile_pool(name="pp_mm", bufs=8, space="PSUM"))

    qkv_tiles = {}
    ex_tiles = {}
    y_tiles = {}

    def emit_P(h):
        # projections: Q,K natural (l, c); V^T (c, l) with ones cols
        xh = xth[h]
        q = q_pool.tile([P, KC, C], BF16)
        k = k_pool.tile([P, KC, C], BF16)
        vt = vt_pool.tile([P, KC, LH + 2], BF16)
        nc.gpsimd.memset(vt[:, :, 0:2], 1.0)
        for m in range(KC):  # l' strips of 128
            pq = pp_mm.tile([P, C], F32, tag="mm")
            for ko in range(KC):
                nc.tensor.matmul(pq[:], xh[:, ko, bass.ts(m, P)],
                                 wq[:, ko, :],
                                 start=(ko == 0), stop=(ko == KC - 1))
            nc.vector.tensor_copy(q[:, m, :], pq[:])
        for m in range(KC):
            pk = pp_mm.tile([P, C], F32, tag="mm")
            for ko in range(KC):
                nc.tensor.matmul(pk[:], xh[:, ko, bass.ts(m, P)],
                                 wk[:, ko, :],
                                 start=(ko == 0), stop=(ko == KC - 1))
            nc.vector.tensor_copy(k[:, m, :], pk[:])
        for m in range(KC):  # c_v strips of 128
            pv = pp_mm.tile([P, LH], F32, tag="mm")
            for ko in range(KC):
                nc.tensor.matmul(pv[:], wv[:, ko, bass.ts(m, P)],
                                 xh[:, ko, :],
                                 start=(ko == 0), stop=(ko == KC - 1))
            nc.scalar.copy(vt[:, m, 2:LH + 2], pv[:])
        qkv_tiles[h] = (q, k, vt)

    def emit_S(h):
        # scores transposed + exp: S^T[d, c] = sum_l K[l,d] Q[l,c]
        q, k, _ = qkv_tiles[h]
        ex = exp_pool.tile([P, KC, C], BF16)
        for ds_ in range(KC):  # d strips of 128
            ps = pp_mm.tile([P, C], F32, tag="mm")
            for m in range(KC):  # contraction over l' chunks
                nc.tensor.matmul(ps[:], k[:, m, bass.ts(ds_, P)],
                                 q[:, m, :],
                                 start=(m == 0), stop=(m == KC - 1))
            nc.scalar.activation(ex[:, ds_, :], ps[:], EXP)
        ex_tiles[h] = ex

    def emit_A(h):
        # AV with fused denominator (rhs cols 0,1 of py1 are ones);
        # normalization fused into the Scalar-engine evacuation via
        # activation(Copy, scale=1/denom per partition).
        _, _, vt = qkv_tiles[h]
        ex = ex_tiles[h]
        y = y_pool.tile([P, KC, LH], BF16)
        for cs in range(KC):  # c strips of 128
            py1 = pp_mm.tile([P, NY1], F32, tag="mm")
            py2 = pp_mm.tile([P, LH + 2 - NY1], F32, tag="mm")
            for ko in range(KC):  # contraction over d chunks
                nc.tensor.matmul(py1[:], ex[:, ko, bass.ts(cs, P)],
                                 vt[:, ko, 0:NY1],
                                 start=(ko == 0), stop=(ko == KC - 1))
            for ko in range(KC):
                nc.tensor.matmul(py2[:], ex[:, ko, bass.ts(cs, P)],
                                 vt[:, ko, NY1:LH + 2],
                                 start=(ko == 0), stop=(ko == KC - 1))
            rc = recip_pool.tile([P, 1], F32)
            nc.vector.reciprocal(rc[:], py1[:, 0:1])
            # split the normalize-evacuation across both engines so the
            # last head's y is ready ~2x sooner for the out-projection
            nc.scalar.mul(y[:, cs, 0:NY1 - 2], py1[:, 2:NY1], rc[:])
            nc.vector.tensor_scalar_mul(y[:, cs, NY1 - 2:LH], py2[:], rc[:])
        y_tiles[h] = y

    def emit_O(h):
        # out projection: out[l, co] = sum_c y^T[c, l] woutT[c, co]
        y = y_tiles[h]
        for m in range(KC):  # l' strips of 128
            po = pp_mm.tile([P, C], F32, tag="mm")
            for ko in range(KC):
                nc.tensor.matmul(po[:], y[:, ko, bass.ts(m, P)],
                                 wout[:, ko, :],
                                 start=(ko == 0), stop=(ko == KC - 1))
            ot = out_pool.tile([P, C], BF16)
            if m % 2 == 0:
                nc.vector.tensor_copy(ot[:], po[:])
                nc.sync.dma_start(out[bass.ds(h * LH + m * P, P), :], ot[:])
            else:
                nc.scalar.copy(ot[:], po[:])
                nc.scalar.dma_start(out[bass.ds(h * LH + m * P, P), :], ot[:])

    # software pipeline: exp/evac latency of head h hides under head h+1
    emit_P(0)
    emit_S(0)
    for h in range(1, HEADS):
        emit_P(h)
        emit_A(h - 1)
        emit_S(h)
        emit_O(h - 1)
    emit_A(HEADS - 1)
    emit_O(HEADS - 1)


def _build_program():
    nc = bass.Bass(trn_type="TRN2", target_bir_lowering=False, debug=False,
                   num_devices=N_CORES)
    xt = nc.dram_tensor("xt", [P, HEADS, KC, LH], BF16,
                        kind="ExternalInput").ap()
    wq_t = nc.dram_tensor("wq_t", [P, KC, C], BF16, kind="ExternalInput").ap()
    wk_t = nc.dram_tensor("wk_t", [P, KC, C], BF16, kind="ExternalInput").ap()
    wv_t = nc.dram_tensor("wv_t", [P, KC, C], BF16, kind="ExternalInput").ap()
    wout_t = nc.dram_tensor("wout_t", [P, KC, C], BF16,
                            kind="ExternalInput").ap()
    out = nc.dram_tensor("out", [L, C], BF16, kind="ExternalOutput").ap()

    from contextlib import ExitStack
    with tile.TileContext(nc) as tc:
        with ExitStack() as ctx:
            _emit(ctx, tc, xt, wq_t, wk_t, wv_t, wout_t, out)
    _split_wide_waits(nc)
    return nc


def _part_major(w_t):
    """(C, C_out) -> (P, KC, C_out): row ko*P+p lands at [p, ko, :] so each
    DMA partition line is one contiguous 4KB chunk."""
    return np.ascontiguousarray(
        w_t.reshape(KC, P, C).transpose(1, 0, 2)).astype(NPBF16)


def _host_inputs(x, w_qkv, w_out):
    """Per-core input maps. Permute l so head h owns rows [h*512, (h+1)*512)
    (original row i*8+h -> permuted row h*512+i), transpose to (c, l), then
    rearrange everything partition-major for single-descriptor DMA lines."""
    wqkv_t = np.ascontiguousarray(w_qkv.T).astype(np.float32)  # (c, 3c)
    wq_t = _part_major(wqkv_t[:, 0:C] * SCALE)
    wk_t = _part_major(wqkv_t[:, C:2 * C])
    wv_t = _part_major(wqkv_t[:, 2 * C:3 * C])
    wout_t = _part_major(w_out.T.astype(np.float32))
    in_maps = []
    for b in range(B):
        xb = x[b]  # (L, C); row l = i*8 + h
        x_perm = xb.reshape(LH, HEADS, C).transpose(1, 0, 2).reshape(L, C)
        xt = np.ascontiguousarray(x_perm.T)  # (c, l)
        # (ko*P+p, h*LH+l') -> (p, h, ko, l')
        xt_hw = np.ascontiguousarray(
            xt.reshape(KC, P, HEADS, LH).transpose(1, 2, 0, 3)).astype(NPBF16)
        in_maps.append({"xt": xt_hw, "wq_t": wq_t, "wk_t": wk_t,
                        "wv_t": wv_t, "wout_t": wout_t})
    return in_maps


def _unpermute(out_perm):
    """(L, C) with rows grouped by head -> original row order i*8+h."""
    return out_perm.reshape(HEADS, LH, C).transpose(1, 0, 2).reshape(L, C)


def kernel(x, w_qkv, w_out, b_out, _run_kwargs=None):
    x = np.asarray(x, dtype=np.float32)
    w_qkv = np.asarray(w_qkv, dtype=np.float32)
    w_out = np.asarray(w_out, dtype=np.float32)
    b_out = np.asarray(b_out, dtype=np.float32)

    nc = _build_program()
    in_maps = _host_inputs(x, w_qkv, w_out)
    res = run_bass_kernel_spmd(nc, in_maps, list(range(N_CORES)),
                               **(_run_kwargs or {}))
    out = np.empty((B, L, C), dtype=np.float32)
    for b in range(B):
        out[b] = _unpermute(res.results[b]["out"].astype(np.float32))
    out += b_out
    if _run_kwargs:
        kernel.last_result = res
    return out


# revision 19
# speedup vs baseline: 1.0253x; 1.0042x over previous
"""Channel-attention transformer block on 8 Trainium2 NeuronCores.

Reference semantics (b=8, l=4096, c=512, h=8 heads carved from the
*sequence* axis, head_pos = l % 8):
    qkv = x @ w_qkv.T ; split q,k,v per head  (each (lh=512, c=512))
    attn = softmax((q.T @ k) / 8, axis=-1)    # (c, c) channel attention
    y.T  = attn @ v.T                         # (c, lh)
    out  = y @ w_out.T + b_out
Sharding: data-parallel over batch - core i handles batch i.

Layout (same math as the fp32r baseline, see kernel_baseline.py):
the l axis is permuted on the host so each head's 512 rows are
contiguous, x is shipped transposed (c, l); Q,K come out of matmuls in
natural (l, c) layout and V^T in (c, l); scores are computed transposed
(S^T = K^T Q) so the softmax sum lands on the partition dim where a
ones-column glued onto the AV rhs computes it; 1/denominator is fused
into the AV evacuation; the out-projection consumes y^T directly.

Perf changes vs the baseline (205 us):
- bf16 matmul path end to end (fp32 PSUM): enables FWL fast weight
  loads (hidden LDWEIGHTS), halves input DMA bytes, bf16 out + host
  upcast halves output DMA.
- PSUM evacuations split across Vector AND Scalar engines (both ~95
  G elem/s on PSUM reads; baseline put ~everything on Vector): q,k and
  out-copies on Vector; v, exp, and the normalize-fused y evacuation
  (activation Copy with per-partition scale=1/denom) on Scalar.
- 3 DMA rails (sync+scalar HWDGE, gpsimd SWDGE) with critical-first
  order: head0 x and wq land in parallel, first matmul ~5 us earlier.
- ~20 warm-up matmuls on a zeroed scratch tile right after the
  preamble: the PE HAM clock-gate un-throttles (1.2->2.4 GHz) during
  the input DMA instead of 20+ us into real work.
- Head stages software-pipelined (P=proj, S=scores, A=AV, O=out-proj
  emitted as P0 S0 | P1 A0 S1 O0 | ...) so exp/evacuation latency of
  head h hides under projections of head h+1 and the PE never stalls.
"""

import numpy as np
import ml_dtypes

import concourse.bass as bass
import concourse.mybir as mybir
import concourse.tile as tile
from concourse.bass_utils import run_bass_kernel_spmd

B = 8
L = 4096
C = 512
HEADS = 8
LH = L // HEADS  # 512
SCALE = 64 ** -0.5  # DIM_HEAD ** -0.5 from the reference
N_CORES = 8
P = 128
KC = C // P  # 4 contraction chunks of 128
F32 = mybir.dt.float32
BF16 = mybir.dt.bfloat16
NPBF16 = ml_dtypes.bfloat16

WARM_MMS = 35   # PE warm-up dummies: bridge preamble-end (~7.8us) to
                # first-data (~15.7us) so HAM stays un-throttled throughout
WARM_N = 256
NY1 = 258  # AV split: 2 denominator cols + 256 v cols (psum bank = 512 fp32)


def _split_wide_waits(nc, max_waits=1):
    """This container's walrus build rejects instructions carrying more than
    ~1 sync wait ("Too many sync wait commands", e.g. in the S3_LW lowering
    of a fused matmul). Hoist surplus waits onto same-engine nops inserted
    immediately before the offending instruction - the engine stalls at the
    same point in its stream, so scheduling semantics are unchanged."""
    for f in nc.m.functions:
        for bb in f.blocks:
            snapshot = list(bb.instructions)
            if not any(
                inst.sync_info and inst.sync_info.on_wait
                and len(inst.sync_info.on_wait) > max_waits
                for inst in snapshot
            ):
                continue
            new = []
            for inst in snapshot:
                si = inst.sync_info
                waits = list(si.on_wait) if si and si.on_wait else []
                if len(waits) > max_waits:
                    for w in waits[:-max_waits]:
                        nop = nc.engines[inst.engine].nop(nofuse=True).ins
                        cur = nc.cur_bb.bb.instructions
                        assert cur[-1] is nop
                        cur.pop()  # re-homed below, right before `inst`
                        nop.sync_info = mybir.SyncInfo(on_wait=[w], on_update=[])
                        new.append(nop)
                    inst.sync_info = mybir.SyncInfo(
                        on_wait=waits[-max_waits:],
                        on_update=list(si.on_update) if si.on_update else [],
                    )
                new.append(inst)
            bb.instructions = new


def _emit(ctx, tc, xt, wq_t, wk_t, wv_t, wout_t, out):
    """Per-core program. DRAM APs: xt (C, L) bf16, wq_t/wk_t/wv_t/wout_t
    (C, C) bf16 (q pre-scaled), out (L, C) bf16."""
    nc = tc.nc
    EXP = mybir.ActivationFunctionType.Exp

    # all DRAM inputs are host-prearranged to partition-major layouts so
    # every DMA coalesces to 128 descriptors of 4KB (descriptor count, not
    # bytes, is what gates the DGE rails: ~33ns/desc HWDGE, ~5ns/desc SWDGE)
    xt_r = xt      # (P, HEADS, KC, LH)
    wq_r = wq_t    # (P, KC, C)
    wk_r = wk_t
    wv_r = wv_t
    wout_r = wout_t

    consts = ctx.enter_context(tc.tile_pool(name="consts", bufs=1))
    xt_pool = ctx.enter_context(tc.tile_pool(name="xt", bufs=HEADS))
    q_pool = ctx.enter_context(tc.tile_pool(name="q", bufs=2))
    k_pool = ctx.enter_context(tc.tile_pool(name="k", bufs=2))
    vt_pool = ctx.enter_context(tc.tile_pool(name="vt", bufs=2))
    exp_pool = ctx.enter_context(tc.tile_pool(name="exp", bufs=2))
    y_pool = ctx.enter_context(tc.tile_pool(name="y", bufs=2))
    out_pool = ctx.enter_context(tc.tile_pool(name="out", bufs=8))
    recip_pool = ctx.enter_context(tc.tile_pool(name="recip", bufs=8))

    wq = consts.tile([P, KC, C], BF16)
    wk = consts.tile([P, KC, C], BF16)
    wv = consts.tile([P, KC, C], BF16)
    wout = consts.tile([P, KC, C], BF16)
    warm = consts.tile([P, WARM_N], BF16)

    xth = [xt_pool.tile([P, KC, LH], BF16, tag="xth", name=f"xth{h}")
           for h in range(HEADS)]

    # --- startup DMA. wq (needed first) split across the two HWDGE rails;
    # the SWDGE/gpsimd rail streams xth0 + the other weights in consumption
    # order; late x heads ride the HWDGE rails where there is slack.
    # Descriptor lines capped at 2KB (1024 bf16) - 4KB lines hard-fault the
    # device (NRT_EXEC_UNIT_UNRECOVERABLE). ---
    MD = dict(max_dma_last_dim=1024)
    nc.gpsimd.dma_start(wq[:], wq_r[:], **MD)
    nc.sync.dma_start(xth[0][:, 0:2, :], xt_r[:, 0, 0:2, :], **MD)
    nc.scalar.dma_start(xth[0][:, 2:KC, :], xt_r[:, 0, 2:KC, :], **MD)
    nc.vector.memset(warm[:], 0.0)
    nc.gpsimd.dma_start(wk[:], wk_r[:], **MD)
    nc.gpsimd.dma_start(wv[:], wv_r[:], **MD)
    nc.gpsimd.dma_start(xth[1][:], xt_r[:, 1, :, :], **MD)
    nc.gpsimd.dma_start(wout[:], wout_r[:], **MD)
    nc.gpsimd.dma_start(xth[2][:], xt_r[:, 2, :, :], **MD)
    nc.gpsimd.dma_start(xth[3][:], xt_r[:, 3, :, :], **MD)
    nc.sync.dma_start(xth[4][:], xt_r[:, 4, :, :], **MD)
    nc.scalar.dma_start(xth[5][:], xt_r[:, 5, :, :], **MD)
    nc.sync.dma_start(xth[6][:], xt_r[:, 6, :, :], **MD)
    nc.scalar.dma_start(xth[7][:], xt_r[:, 7, :, :], **MD)

    # --- PE warm-up: dummy matmuls on zeros keep the PE busy from the
    # end of the preamble so the HAM clock-gate lifts (1.2 -> 2.4 GHz)
    # before the real matmuls start. Own PSUM pool, closed before pp_mm
    # opens so no bank is wasted. ---
    with tc.tile_pool(name="warmp", bufs=1, space="PSUM") as wpool:
        wp = wpool.tile([P, WARM_N], F32)
        for _ in range(WARM_MMS):
            nc.tensor.matmul(wp[:], warm[:, 0:P], warm[:],
                             start=True, stop=True)

    pp_mm = ctx.enter_context(tc.tile_pool(name="pp_mm", bufs=8, space="PSUM"))

    qkv_tiles = {}
    ex_tiles = {}
    y_tiles = {}

    def emit_P(h):
        # projections: Q,K natural (l, c); V^T (c, l) with ones cols
        xh = xth[h]
        q = q_pool.tile([P, KC, C], BF16)
        k = k_pool.tile([P, KC, C], BF16)
        vt = vt_pool.tile([P, KC, LH + 2], BF16)
        nc.gpsimd.memset(vt[:, :, 0:2], 1.0)
        for m in range(KC):  # l' strips of 128
            pq = pp_mm.tile([P, C], F32, tag="mm")
            for ko in range(KC):
                nc.tensor.matmul(pq[:], xh[:, ko, bass.ts(m, P)],
                                 wq[:, ko, :],
                                 start=(ko == 0), stop=(ko == KC - 1))
            nc.vector.tensor_copy(q[:, m, :], pq[:])
        for m in range(KC):
            pk = pp_mm.tile([P, C], F32, tag="mm")
            for ko in range(KC):
                nc.tensor.matmul(pk[:], xh[:, ko, bass.ts(m, P)],
                                 wk[:, ko, :],
                                 start=(ko == 0), stop=(ko == KC - 1))
            nc.vector.tensor_copy(k[:, m, :], pk[:])
        for m in range(KC):  # c_v strips of 128
            pv = pp_mm.tile([P, LH], F32, tag="mm")
            for ko in range(KC):
                nc.tensor.matmul(pv[:], wv[:, ko, bass.ts(m, P)],
                                 xh[:, ko, :],
                                 start=(ko == 0), stop=(ko == KC - 1))
            nc.scalar.copy(vt[:, m, 2:LH + 2], pv[:])
        qkv_tiles[h] = (q, k, vt)

    def emit_S(h):
        # scores transposed + exp: S^T[d, c] = sum_l K[l,d] Q[l,c]
        q, k, _ = qkv_tiles[h]
        ex = exp_pool.tile([P, KC, C], BF16)
        for ds_ in range(KC):  # d strips of 128
            ps = pp_mm.tile([P, C], F32, tag="mm")
            for m in range(KC):  # contraction over l' chunks
                nc.tensor.matmul(ps[:], k[:, m, bass.ts(ds_, P)],
                                 q[:, m, :],
                                 start=(m == 0), stop=(m == KC - 1))
            nc.scalar.activation(ex[:, ds_, :], ps[:], EXP)
        ex_tiles[h] = ex

    def emit_A(h):
        # AV with fused denominator (rhs cols 0,1 of py1 are ones);
        # normalization fused into the Scalar-engine evacuation via
        # activation(Copy, scale=1/denom per partition).
        _, _, vt = qkv_tiles[h]
        ex = ex_tiles[h]
        y = y_pool.tile([P, KC, LH], BF16)
        for cs in range(KC):  # c strips of 128
            py1 = pp_mm.tile([P, NY1], F32, tag="mm")
            py2 = pp_mm.tile([P, LH + 2 - NY1], F32, tag="mm")
            for ko in range(KC):  # contraction over d chunks
                nc.tensor.matmul(py1[:], ex[:, ko, bass.ts(cs, P)],
                                 vt[:, ko, 0:NY1],
                                 start=(ko == 0), stop=(ko == KC - 1))
            for ko in range(KC):
                nc.tensor.matmul(py2[:], ex[:, ko, bass.ts(cs, P)],
                                 vt[:, ko, NY1:LH + 2],
                                 start=(ko == 0), stop=(ko == KC - 1))
            rc = recip_pool.tile([P, 1], F32)
            nc.vector.reciprocal(rc[:], py1[:, 0:1])
            # split the normalize-evacuation across both engines so the
            # last head's y is ready ~2x sooner for the out-projection
            nc.scalar.mul(y[:, cs, 0:NY1 - 2], py1[:, 2:NY1], rc[:])
            nc.vector.tensor_scalar_mul(y[:, cs, NY1 - 2:LH], py2[:], rc[:])
        y_tiles[h] = y

    def emit_O(h):
        # out projection: out[l, co] = sum_c y^T[c, l] woutT[c, co]
        y = y_tiles[h]
        for m in range(KC):  # l' strips of 128
            po = pp_mm.tile([P, C], F32, tag="mm")
            for ko in range(KC):
                nc.tensor.matmul(po[:], y[:, ko, bass.ts(m, P)],
                                 wout[:, ko, :],
                                 start=(ko == 0), stop=(ko == KC - 1))
            ot = out_pool.tile([P, C], BF16)
            if m % 2 == 0:
                nc.vector.tensor_copy(ot[:], po[:])
                nc.sync.dma_start(out[bass.ds(h * LH + m * P, P), :], ot[:])
            else:
                nc.scalar.copy(ot[:], po[:])
                nc.scalar.dma_start(out[bass.ds(h * LH + m * P, P), :], ot[:])

    # software pipeline: exp/evac latency of head h hides under head h+1
    emit_P(0)
    emit_S(0)
    for h in range(1, HEADS):
        emit_P(h)
        emit_A(h - 1)
        emit_S(h)
        emit_O(h - 1)
    emit_A(HEADS - 1)
    emit_O(HEADS - 1)


def _build_program():
    nc = bass.Bass(trn_type="TRN2", target_bir_lowering=False, debug=False,
                   num_devices=N_CORES)
    xt = nc.dram_tensor("xt", [P, HEADS, KC, LH], BF16,
                        kind="ExternalInput").ap()
    wq_t = nc.dram_tensor("wq_t", [P, KC, C], BF16, kind="ExternalInput").ap()
    wk_t = nc.dram_tensor("wk_t", [P, KC, C], BF16, kind="ExternalInput").ap()
    wv_t = nc.dram_tensor("wv_t", [P, KC, C], BF16, kind="ExternalInput").ap()
    wout_t = nc.dram_tensor("wout_t", [P, KC, C], BF16,
                            kind="ExternalInput").ap()
    out = nc.dram_tensor("out", [L, C], BF16, kind="ExternalOutput").ap()

    from contextlib import ExitStack
    with tile.TileContext(nc) as tc:
        with ExitStack() as ctx:
            _emit(ctx, tc, xt, wq_t, wk_t, wv_t, wout_t, out)
    _split_wide_waits(nc)
    return nc


def _part_major(w_t):
    """(C, C_out) -> (P, KC, C_out): row ko*P+p lands at [p, ko, :] so each
    DMA partition line is one contiguous 4KB chunk."""
    return np.ascontiguousarray(
        w_t.reshape(KC, P, C).transpose(1, 0, 2)).astype(NPBF16)


def _host_inputs(x, w_qkv, w_out):
    """Per-core input maps. Permute l so head h owns rows [h*512, (h+1)*512)
    (original row i*8+h -> permuted row h*512+i), transpose to (c, l), then
    rearrange everything partition-major for single-descriptor DMA lines."""
    wqkv_t = np.ascontiguousarray(w_qkv.T).astype(np.float32)  # (c, 3c)
    wq_t = _part_major(wqkv_t[:, 0:C] * SCALE)
    wk_t = _part_major(wqkv_t[:, C:2 * C])
    wv_t = _part_major(wqkv_t[:, 2 * C:3 * C])
    wout_t = _part_major(w_out.T.astype(np.float32))
    in_maps = []
    for b in range(B):
        xb = x[b]  # (L, C); row l = i*8 + h
        x_perm = xb.reshape(LH, HEADS, C).transpose(1, 0, 2).reshape(L, C)
        xt = np.ascontiguousarray(x_perm.T)  # (c, l)
        # (ko*P+p, h*LH+l') -> (p, h, ko, l')
        xt_hw = np.ascontiguousarray(
            xt.reshape(KC, P, HEADS, LH).transpose(1, 2, 0, 3)).astype(NPBF16)
        in_maps.append({"xt": xt_hw, "wq_t": wq_t, "wk_t": wk_t,
                        "wv_t": wv_t, "wout_t": wout_t})
    return in_maps


def _unpermute(out_perm):
    """(L, C) with rows grouped by head -> original row order i*8+h."""
    return out_perm.reshape(HEADS, LH, C).transpose(1, 0, 2).reshape(L, C)


def kernel(x, w_qkv, w_out, b_out, _run_kwargs=None):
    x = np.asarray(x, dtype=np.float32)
    w_qkv = np.asarray(w_qkv, dtype=np.float32)
    w_out = np.asarray(w_out, dtype=np.float32)
    b_out = np.asarray(b_out, dtype=np.float32)

    nc = _build_program()
    in_maps = _host_inputs(x, w_qkv, w_out)
    res = run_bass_kernel_spmd(nc, in_maps, list(range(N_CORES)),
                               **(_run_kwargs or {}))
    out = np.empty((B, L, C), dtype=np.float32)
    for b in range(B):
        out[b] = _unpermute(res.results[b]["out"].astype(np.float32))
    out += b_out
    if _run_kwargs:
        kernel.last_result = res
    return out


# revision 24
# speedup vs baseline: 1.0258x; 1.0005x over previous
"""Channel-attention transformer block on 8 Trainium2 NeuronCores.

Reference semantics (b=8, l=4096, c=512, h=8 heads carved from the
*sequence* axis, head_pos = l % 8):
    qkv = x @ w_qkv.T ; split q,k,v per head  (each (lh=512, c=512))
    attn = softmax((q.T @ k) / 8, axis=-1)    # (c, c) channel attention
    y.T  = attn @ v.T                         # (c, lh)
    out  = y @ w_out.T + b_out
Sharding: data-parallel over batch - core i handles batch i.

Layout (same math as the fp32r baseline, see kernel_baseline.py):
the l axis is permuted on the host so each head's 512 rows are
contiguous, x is shipped transposed (c, l); Q,K come out of matmuls in
natural (l, c) layout and V^T in (c, l); scores are computed transposed
(S^T = K^T Q) so the softmax sum lands on the partition dim where a
ones-column glued onto the AV rhs computes it; 1/denominator is fused
into the AV evacuation; the out-projection consumes y^T directly.

Perf changes vs the baseline (205 us):
- bf16 matmul path end to end (fp32 PSUM): enables FWL fast weight
  loads (hidden LDWEIGHTS), halves input DMA bytes, bf16 out + host
  upcast halves output DMA.
- PSUM evacuations split across Vector AND Scalar engines (both ~95
  G elem/s on PSUM reads; baseline put ~everything on Vector): q,k and
  out-copies on Vector; v, exp, and the normalize-fused y evacuation
  (activation Copy with per-partition scale=1/denom) on Scalar.
- 3 DMA rails (sync+scalar HWDGE, gpsimd SWDGE) with critical-first
  order: head0 x and wq land in parallel, first matmul ~5 us earlier.
- ~20 warm-up matmuls on a zeroed scratch tile right after the
  preamble: the PE HAM clock-gate un-throttles (1.2->2.4 GHz) during
  the input DMA instead of 20+ us into real work.
- Head stages software-pipelined (P=proj, S=scores, A=AV, O=out-proj
  emitted as P0 S0 | P1 A0 S1 O0 | ...) so exp/evacuation latency of
  head h hides under projections of head h+1 and the PE never stalls.
"""

import numpy as np
import ml_dtypes

import concourse.bass as bass
import concourse.mybir as mybir
import concourse.tile as tile
from concourse.bass_utils import run_bass_kernel_spmd

B = 8
L = 4096
C = 512
HEADS = 8
LH = L // HEADS  # 512
SCALE = 64 ** -0.5  # DIM_HEAD ** -0.5 from the reference
N_CORES = 8
P = 128
KC = C // P  # 4 contraction chunks of 128
F32 = mybir.dt.float32
BF16 = mybir.dt.bfloat16
NPBF16 = ml_dtypes.bfloat16

WARM_MMS = 35   # PE warm-up dummies: bridge preamble-end (~7.8us) to
                # first-data (~15.7us) so HAM stays un-throttled throughout
WARM_N = 256
NY1 = 258  # AV split: 2 denominator cols + 256 v cols (psum bank = 512 fp32)


def _split_wide_waits(nc, max_waits=1):
    """This container's walrus build rejects instructions carrying more than
    ~1 sync wait ("Too many sync wait commands", e.g. in the S3_LW lowering
    of a fused matmul). Hoist surplus waits onto same-engine nops inserted
    immediately before the offending instruction - the engine stalls at the
    same point in its stream, so scheduling semantics are unchanged."""
    for f in nc.m.functions:
        for bb in f.blocks:
            snapshot = list(bb.instructions)
            if not any(
                inst.sync_info and inst.sync_info.on_wait
                and len(inst.sync_info.on_wait) > max_waits
                for inst in snapshot
            ):
                continue
            new = []
            for inst in snapshot:
                si = inst.sync_info
                waits = list(si.on_wait) if si and si.on_wait else []
                if len(waits) > max_waits:
                    for w in waits[:-max_waits]:
                        nop = nc.engines[inst.engine].nop(nofuse=True).ins
                        cur = nc.cur_bb.bb.instructions
                        assert cur[-1] is nop
                        cur.pop()  # re-homed below, right before `inst`
                        nop.sync_info = mybir.SyncInfo(on_wait=[w], on_update=[])
                        new.append(nop)
                    inst.sync_info = mybir.SyncInfo(
                        on_wait=waits[-max_waits:],
                        on_update=list(si.on_update) if si.on_update else [],
                    )
                new.append(inst)
            bb.instructions = new


def _emit(ctx, tc, xt, wq_t, wk_t, wv_t, wout_t, out):
    """Per-core program. DRAM APs: xt (C, L) bf16, wq_t/wk_t/wv_t/wout_t
    (C, C) bf16 (q pre-scaled), out (L, C) bf16."""
    nc = tc.nc
    EXP = mybir.ActivationFunctionType.Exp

    # all DRAM inputs are host-prearranged to partition-major layouts so
    # every DMA coalesces to 128 descriptors of 4KB (descriptor count, not
    # bytes, is what gates the DGE rails: ~33ns/desc HWDGE, ~5ns/desc SWDGE)
    xt_r = xt      # (P, HEADS, KC, LH)
    wq_r = wq_t    # (P, KC, C)
    wk_r = wk_t
    wv_r = wv_t
    wout_r = wout_t

    consts = ctx.enter_context(tc.tile_pool(name="consts", bufs=1))
    xt_pool = ctx.enter_context(tc.tile_pool(name="xt", bufs=HEADS))
    q_pool = ctx.enter_context(tc.tile_pool(name="q", bufs=2))
    k_pool = ctx.enter_context(tc.tile_pool(name="k", bufs=2))
    vt_pool = ctx.enter_context(tc.tile_pool(name="vt", bufs=3))
    exp_pool = ctx.enter_context(tc.tile_pool(name="exp", bufs=2))
    y_pool = ctx.enter_context(tc.tile_pool(name="y", bufs=2))
    out_pool = ctx.enter_context(tc.tile_pool(name="out", bufs=8))
    recip_pool = ctx.enter_context(tc.tile_pool(name="recip", bufs=8))

    wq = consts.tile([P, KC, C], BF16)
    wk = consts.tile([P, KC, C], BF16)
    wv = consts.tile([P, KC, C], BF16)
    wout = consts.tile([P, KC, C], BF16)
    warm = consts.tile([P, WARM_N], BF16)

    xth = [xt_pool.tile([P, KC, LH], BF16, tag="xth", name=f"xth{h}")
           for h in range(HEADS)]

    # --- startup DMA. wq (needed first) split across the two HWDGE rails;
    # the SWDGE/gpsimd rail streams xth0 + the other weights in consumption
    # order; late x heads ride the HWDGE rails where there is slack.
    # Descriptor lines capped at 2KB (1024 bf16) - 4KB lines hard-fault the
    # device (NRT_EXEC_UNIT_UNRECOVERABLE). ---
    MD = dict(max_dma_last_dim=1024)
    nc.gpsimd.dma_start(wq[:], wq_r[:], **MD)
    nc.sync.dma_start(xth[0][:, 0:2, :], xt_r[:, 0, 0:2, :], **MD)
    nc.scalar.dma_start(xth[0][:, 2:KC, :], xt_r[:, 0, 2:KC, :], **MD)
    nc.vector.memset(warm[:], 0.0)
    nc.gpsimd.dma_start(xth[1][:], xt_r[:, 1, :, :], **MD)
    nc.gpsimd.dma_start(wk[:], wk_r[:], **MD)
    nc.gpsimd.dma_start(wv[:], wv_r[:], **MD)
    nc.gpsimd.dma_start(wout[:], wout_r[:], **MD)
    nc.gpsimd.dma_start(xth[2][:], xt_r[:, 2, :, :], **MD)
    nc.gpsimd.dma_start(xth[3][:], xt_r[:, 3, :, :], **MD)
    nc.sync.dma_start(xth[4][:], xt_r[:, 4, :, :], **MD)
    nc.scalar.dma_start(xth[5][:], xt_r[:, 5, :, :], **MD)
    nc.sync.dma_start(xth[6][:], xt_r[:, 6, :, :], **MD)
    nc.scalar.dma_start(xth[7][:], xt_r[:, 7, :, :], **MD)

    # --- PE warm-up: dummy matmuls on zeros keep the PE busy from the
    # end of the preamble so the HAM clock-gate lifts (1.2 -> 2.4 GHz)
    # before the real matmuls start. Own PSUM pool, closed before pp_mm
    # opens so no bank is wasted. ---
    with tc.tile_pool(name="warmp", bufs=1, space="PSUM") as wpool:
        wp = wpool.tile([P, WARM_N], F32)
        for _ in range(WARM_MMS):
            nc.tensor.matmul(wp[:], warm[:, 0:P], warm[:],
                             start=True, stop=True)

    pp_mm = ctx.enter_context(tc.tile_pool(name="pp_mm", bufs=8, space="PSUM"))

    qkv_tiles = {}
    ex_tiles = {}
    y_tiles = {}

    def emit_Pq(h):
        # q projection: natural (l, c) layout
        xh = xth[h]
        q = q_pool.tile([P, KC, C], BF16, name="q", tag="q")
        for m in range(KC):  # l' strips of 128
            pq = pp_mm.tile([P, C], F32, tag="mm")
            for ko in range(KC):
                nc.tensor.matmul(pq[:], xh[:, ko, bass.ts(m, P)],
                                 wq[:, ko, :],
                                 start=(ko == 0), stop=(ko == KC - 1))
            nc.vector.tensor_copy(q[:, m, :], pq[:])
        qkv_tiles[h] = [q, None, None]

    def emit_Pk(h):
        xh = xth[h]
        k = k_pool.tile([P, KC, C], BF16, name="k", tag="k")
        for m in range(KC):
            pk = pp_mm.tile([P, C], F32, tag="mm")
            for ko in range(KC):
                nc.tensor.matmul(pk[:], xh[:, ko, bass.ts(m, P)],
                                 wk[:, ko, :],
                                 start=(ko == 0), stop=(ko == KC - 1))
            nc.vector.tensor_copy(k[:, m, :], pk[:])
        qkv_tiles[h][1] = k

    def emit_Pv(h):
        # V^T (c, l) with ones cols glued in front for the denominator
        xh = xth[h]
        vt = vt_pool.tile([P, KC, LH + 2], BF16, name="vt", tag="vt")
        nc.gpsimd.memset(vt[:, :, 0:2], 1.0)
        for m in range(KC):  # c_v strips of 128
            pv = pp_mm.tile([P, LH], F32, tag="mm")
            for ko in range(KC):
                nc.tensor.matmul(pv[:], wv[:, ko, bass.ts(m, P)],
                                 xh[:, ko, :],
                                 start=(ko == 0), stop=(ko == KC - 1))
            nc.scalar.copy(vt[:, m, 2:LH + 2], pv[:])
        qkv_tiles[h][2] = vt

    def emit_P(h):
        emit_Pq(h)
        emit_Pk(h)
        emit_Pv(h)

    def emit_S(h):
        # scores transposed + exp: S^T[d, c] = sum_l K[l,d] Q[l,c]
        q, k, _ = qkv_tiles[h]
        ex = exp_pool.tile([P, KC, C], BF16)
        for ds_ in range(KC):  # d strips of 128
            ps = pp_mm.tile([P, C], F32, tag="mm")
            for m in range(KC):  # contraction over l' chunks
                nc.tensor.matmul(ps[:], k[:, m, bass.ts(ds_, P)],
                                 q[:, m, :],
                                 start=(m == 0), stop=(m == KC - 1))
            nc.scalar.activation(ex[:, ds_, :], ps[:], EXP)
        ex_tiles[h] = ex

    def emit_A(h):
        # AV with fused denominator (rhs cols 0,1 of py1 are ones);
        # normalization fused into the Scalar-engine evacuation via
        # activation(Copy, scale=1/denom per partition).
        _, _, vt = qkv_tiles[h]
        ex = ex_tiles[h]
        y = y_pool.tile([P, KC, LH], BF16)
        for cs in range(KC):  # c strips of 128
            py1 = pp_mm.tile([P, NY1], F32, tag="mm")
            py2 = pp_mm.tile([P, LH + 2 - NY1], F32, tag="mm")
            for ko in range(KC):  # contraction over d chunks
                nc.tensor.matmul(py1[:], ex[:, ko, bass.ts(cs, P)],
                                 vt[:, ko, 0:NY1],
                                 start=(ko == 0), stop=(ko == KC - 1))
            for ko in range(KC):
                nc.tensor.matmul(py2[:], ex[:, ko, bass.ts(cs, P)],
                                 vt[:, ko, NY1:LH + 2],
                                 start=(ko == 0), stop=(ko == KC - 1))
            rc = recip_pool.tile([P, 1], F32)
            nc.vector.reciprocal(rc[:], py1[:, 0:1])
            # split the normalize-evacuation across both engines so the
            # last head's y is ready ~2x sooner for the out-projection
            nc.scalar.mul(y[:, cs, 0:NY1 - 2], py1[:, 2:NY1], rc[:])
            nc.vector.tensor_scalar_mul(y[:, cs, NY1 - 2:LH], py2[:], rc[:])
        y_tiles[h] = y

    def emit_O(h):
        # out projection: out[l, co] = sum_c y^T[c, l] woutT[c, co]
        y = y_tiles[h]
        for m in range(KC):  # l' strips of 128
            po = pp_mm.tile([P, C], F32, tag="mm")
            for ko in range(KC):
                nc.tensor.matmul(po[:], y[:, ko, bass.ts(m, P)],
                                 wout[:, ko, :],
                                 start=(ko == 0), stop=(ko == KC - 1))
            ot = out_pool.tile([P, C], BF16)
            if m % 2 == 0:
                nc.vector.tensor_copy(ot[:], po[:])
                nc.sync.dma_start(out[bass.ds(h * LH + m * P, P), :], ot[:])
            else:
                nc.scalar.copy(ot[:], po[:])
                nc.scalar.dma_start(out[bass.ds(h * LH + m * P, P), :], ot[:])

    # software pipeline: exp/evac latency of head h hides under later
    # heads' matmuls. Prologue interleaves heads 0/1 per projection stage
    # so the PE never waits on the wk/wv weight DMAs (only wq + xth0 gate
    # the first matmul).
    emit_Pq(0)
    emit_Pq(1)
    emit_Pk(0)
    emit_Pk(1)
    emit_Pv(0)
    emit_Pv(1)
    emit_S(0)
    for h in range(2, HEADS):
        emit_P(h)
        emit_A(h - 2)
        emit_S(h - 1)
        emit_O(h - 2)
    emit_A(HEADS - 2)
    emit_S(HEADS - 1)
    emit_O(HEADS - 2)
    emit_A(HEADS - 1)
    emit_O(HEADS - 1)


def _build_program():
    nc = bass.Bass(trn_type="TRN2", target_bir_lowering=False, debug=False,
                   num_devices=N_CORES)
    xt = nc.dram_tensor("xt", [P, HEADS, KC, LH], BF16,
                        kind="ExternalInput").ap()
    wq_t = nc.dram_tensor("wq_t", [P, KC, C], BF16, kind="ExternalInput").ap()
    wk_t = nc.dram_tensor("wk_t", [P, KC, C], BF16, kind="ExternalInput").ap()
    wv_t = nc.dram_tensor("wv_t", [P, KC, C], BF16, kind="ExternalInput").ap()
    wout_t = nc.dram_tensor("wout_t", [P, KC, C], BF16,
                            kind="ExternalInput").ap()
    out = nc.dram_tensor("out", [L, C], BF16, kind="ExternalOutput").ap()

    from contextlib import ExitStack
    with tile.TileContext(nc) as tc:
        with ExitStack() as ctx:
            _emit(ctx, tc, xt, wq_t, wk_t, wv_t, wout_t, out)
    _split_wide_waits(nc)
    return nc


def _part_major(w_t):
    """(C, C_out) -> (P, KC, C_out): row ko*P+p lands at [p, ko, :] so each
    DMA partition line is one contiguous 4KB chunk."""
    return np.ascontiguousarray(
        w_t.reshape(KC, P, C).transpose(1, 0, 2)).astype(NPBF16)


def _host_inputs(x, w_qkv, w_out):
    """Per-core input maps. Permute l so head h owns rows [h*512, (h+1)*512)
    (original row i*8+h -> permuted row h*512+i), transpose to (c, l), then
    rearrange everything partition-major for single-descriptor DMA lines."""
    wqkv_t = np.ascontiguousarray(w_qkv.T).astype(np.float32)  # (c, 3c)
    wq_t = _part_major(wqkv_t[:, 0:C] * SCALE)
    wk_t = _part_major(wqkv_t[:, C:2 * C])
    wv_t = _part_major(wqkv_t[:, 2 * C:3 * C])
    wout_t = _part_major(w_out.T.astype(np.float32))
    in_maps = []
    for b in range(B):
        xb = x[b]  # (L, C); row l = i*8 + h
        x_perm = xb.reshape(LH, HEADS, C).transpose(1, 0, 2).reshape(L, C)
        xt = np.ascontiguousarray(x_perm.T)  # (c, l)
        # (ko*P+p, h*LH+l') -> (p, h, ko, l')
        xt_hw = np.ascontiguousarray(
            xt.reshape(KC, P, HEADS, LH).transpose(1, 2, 0, 3)).astype(NPBF16)
        in_maps.append({"xt": xt_hw, "wq_t": wq_t, "wk_t": wk_t,
                        "wv_t": wv_t, "wout_t": wout_t})
    return in_maps


def _unpermute(out_perm):
    """(L, C) with rows grouped by head -> original row order i*8+h."""
    return out_perm.reshape(HEADS, LH, C).transpose(1, 0, 2).reshape(L, C)


def kernel(x, w_qkv, w_out, b_out, _run_kwargs=None):
    x = np.asarray(x, dtype=np.float32)
    w_qkv = np.asarray(w_qkv, dtype=np.float32)
    w_out = np.asarray(w_out, dtype=np.float32)
    b_out = np.asarray(b_out, dtype=np.float32)

    nc = _build_program()
    in_maps = _host_inputs(x, w_qkv, w_out)
    res = run_bass_kernel_spmd(nc, in_maps, list(range(N_CORES)),
                               **(_run_kwargs or {}))
    out = np.empty((B, L, C), dtype=np.float32)
    for b in range(B):
        out[b] = _unpermute(res.results[b]["out"].astype(np.float32))
    out += b_out
    if _run_kwargs:
        kernel.last_result = res
    return out
